# revision 30
# baseline (speedup 1.0000x reference)
"""Trainium2 Bass kernel for nn_Channel: adaptive max-pool(3) -> 16 depthwise
3x3 convs -> sigmoid-sum channel gate -> leaky(gate*x).

Data-parallel over batch: 32 batches -> 4 per core x 8 cores. Weights/biases
replicated. Self-contained: hardcodes shapes from the problem spec.

The kernel is HBM-bandwidth bound (read x once + write out once = 75.5 MB
per core). Default schedule ("fence"): every DMA is issued on the single SP
HWDGE queue (qSPDynamicHW) in [4 loads][4 stores] phase order — FIFO on one
queue keeps the HBM bus direction-pure within each phase (measured: mixed
read/write traffic runs 10-20% slower than phase-separated). Tiny fence DMAs
at each phase boundary re-align the 16 independent SDMA engine rings so fast
engines cannot run ahead into the opposite-direction phase.
"""

import numpy as np

import concourse.bacc as bacc
import concourse.tile as tile
from concourse import mybir
from concourse.bass_utils import run_bass_kernel_spmd

AFT = mybir.ActivationFunctionType
ALU = mybir.AluOpType
F32 = mybir.dt.float32
BF16 = mybir.dt.bfloat16

B, C, H, W = 32, 256, 96, 96
N_CORES = 8
B_SH = B // N_CORES          # 4 batches per core
P = 128                      # SBUF partitions
G = C // P                   # 2 channel groups
HW = H * W                   # 9216
K = 16                       # number of depthwise convs
NEG = 0.01                   # leaky relu slope (torch default)


def build(repeat: int = 1, copy_only: bool = False, mode: str = "full"):
    if copy_only:
        mode = "copy"
    nc = bacc.Bacc(None, num_devices=N_CORES if "barrier" in mode else None)
    x = nc.dram_tensor("x", [B_SH, C, H, W], F32, kind="ExternalInput")
    if "barrier" in mode:
        bar_in = nc.dram_tensor("bar_in", [1, 2], F32)
        bar_out = nc.dram_tensor("bar_out", [1, 2], F32)
    # packed per-channel weights+biases: [p, g, k*9 weights .. k biases]
    wb = nc.dram_tensor("wb", [P, G, K * 9 + K], F32, kind="ExternalInput")
    out = nc.dram_tensor("out", [B_SH, C, H, W], F32, kind="ExternalOutput")

    # channel c = g*128 + p -> partition p of group g
    x2 = x.rearrange("b (g p) h w -> (b g) p (h w)", g=G, p=P)
    o2 = out.rearrange("b (g p) h w -> (b g) p (h w)", g=G, p=P)

    # All DMAs go through the single SP HWDGE queue (qSPDynamicHW) in
    # program order: 4 loads, then 4 stores, alternating. One FIFO queue
    # means the SDMA engines never interleave read and write packets on
    # the HBM bus (measured: mixed-direction traffic runs ~13% slower
    # than phase-separated; load-only 371 GB/s, store-only 379 GB/s).
    PH = 4  # tiles per direction phase; also the xp pool depth (4x36KB/part)
    DEPTH = PH

    with tile.TileContext(nc) as tc:
        with (
            tc.tile_pool(name="xp", bufs=DEPTH) as xp,
            tc.tile_pool(name="cst", bufs=1) as cst,
            tc.tile_pool(name="sm", bufs=4) as sm,
            tc.tile_pool(name="sp8", bufs=8) as sp8,
            tc.tile_pool(name="ab", bufs=8 if mode == "bf16b" else 24) as ab,
        ):
            wb_t = cst.tile([P, G, K * 9 + K], F32)
            nc.sync.dma_start(wb_t[:], wb[:])
            # warmup read so the wb DMA wait lands here, not on the first
            # TensorTensor (whose ISA format has too few sync-wait slots)
            warm = cst.tile([P, 1], F32)
            nc.vector.tensor_copy(warm[:], wb_t[:, 0, 0:1])

            # seq[j] = tile index of the j-th unit of work; repeat>1 re-runs
            # the whole pass (for differential HW timing) writing identical
            # bytes to out each pass.
            seq = [i % (B_SH * G) for i in range(B_SH * G * repeat)]
            xts = {}

            if mode == "store":
                # store-only probe: one tile loaded once, stored everywhere
                st = cst.tile([P, HW], F32)
                nc.scalar.dma_start(st[:], x2[0])
                for i in seq:
                    nc.sync.dma_start(o2[i], st[:])
                seq = []

            if mode == "bf16load":
                # SWDGE cast-load probe: HBM f32 -> SBUF bf16, 24 units/pass
                x3 = x.rearrange(
                    "b (g p) (hb h) w -> b g hb p (h w)", g=G, p=P, hb=3, h=32
                )
                aslots = [cst.tile([P, 3072], BF16, name=f"a{k}") for k in range(6)]
                units = [
                    (b, g, hb) for b in range(B_SH) for g in range(G) for hb in range(3)
                ]
                for r in range(repeat):
                    for u, (b, g, hb) in enumerate(units):
                        nc.gpsimd.dma_start(aslots[u % 6][:], x3[b, g, hb])
                seq = []

            if mode == "bf16":
                # bf16-arena schedule: the whole pass (24 units of
                # [128, 3072]) lives in SBUF as bf16, so each pass is ONE
                # load phase + ONE store phase. Every big transfer is a
                # SWDGE cast DMA (f32 HBM <-> bf16 SBUF) on the single
                # qPoolDynamic queue, whose FIFO order alone enforces
                # direction-pure phases. bf16 rounding of x and of the
                # scaled output adds ~1e-3 rel error (gate <2e-2 tol).
                x3 = x.rearrange(
                    "b (g p) (hb h) w -> b g hb p (h w)", g=G, p=P, hb=3, h=32
                )
                o3 = out.rearrange(
                    "b (g p) (hb h) w -> b g hb p (h w)", g=G, p=P, hb=3, h=32
                )
                units = [
                    (b, g, hb) for b in range(B_SH) for g in range(G) for hb in range(3)
                ]

                def gate_chain(g, pt):
                    # conv[p,k] = sum_j pooled[p,j] * wt[p,k,j] + bias
                    prod = sm.tile([P, K, 9], F32, tag="prod")
                    pooled_b = pt[:].unsqueeze(1).broadcast_to([P, K, 9])
                    wt_v = wb_t[:, g, 0 : K * 9].rearrange("p (k n) -> p k n", k=K)
                    nc.vector.tensor_tensor(prod[:], wt_v, pooled_b, ALU.mult)
                    conv = sm.tile([P, K], F32, tag="conv")
                    nc.vector.reduce_sum(conv[:], prod[:], axis=mybir.AxisListType.X)
                    nc.vector.tensor_add(conv[:], conv[:], wb_t[:, g, K * 9 :])
                    lr = sm.tile([P, K], F32, tag="lr")
                    nc.scalar.activation(lr[:], conv[:], AFT.Lrelu, alpha=NEG)
                    sig = sm.tile([P, K], F32, tag="sig")
                    gate = sm.tile([P, 1], F32, tag="gate")
                    nc.scalar.activation(sig[:], lr[:], AFT.Sigmoid, accum_out=gate[:])
                    s = sp8.tile([P, 1], F32, tag="s")
                    nc.scalar.activation(s[:], gate[:], AFT.Lrelu, alpha=NEG)
                    return s

                for r in range(repeat):
                    arena = {}
                    pooled = {}
                    s_tiles = {}
                    for (b, g, hb) in units:
                        at = ab.tile([P, 3072], BF16, tag="at")
                        arena[(b, g, hb)] = at
                        nc.gpsimd.dma_start(at[:], x3[b, g, hb])
                        if hb == 0:
                            pt = sm.tile([P, 9], F32, tag="pooled")
                            pooled[(b, g)] = pt
                        xv = at[:].rearrange("p (h wb w) -> p wb h w", h=32, wb=3, w=32)
                        nc.vector.reduce_max(
                            pooled[(b, g)][:, hb * 3 : (hb + 1) * 3],
                            xv,
                            axis=mybir.AxisListType.XY,
                        )
                        if hb == 2:
                            s_tiles[(b, g)] = gate_chain(g, pooled[(b, g)])
                    for (b, g, hb) in units:
                        at = arena.pop((b, g, hb))
                        nc.scalar.activation(
                            at[:], at[:], AFT.Lrelu, scale=s_tiles[(b, g)][:], alpha=NEG
                        )
                        nc.gpsimd.dma_start(o3[b, g, hb], at[:])
                seq = []

            if mode == "bf16b":
                # bf16-arena with full-size units: 8 tiles of [128, 9216]
                # bf16 per pass (one SWDGE cast DMA each way per tile, one
                # descriptor per partition row). Same phase structure as
                # "bf16" but a third of the DMA/instruction count.
                def gate_chain_b(g, pt):
                    prod = sm.tile([P, K, 9], F32, tag="prod")
                    pooled_b = pt[:].unsqueeze(1).broadcast_to([P, K, 9])
                    wt_v = wb_t[:, g, 0 : K * 9].rearrange("p (k n) -> p k n", k=K)
                    nc.vector.tensor_tensor(prod[:], wt_v, pooled_b, ALU.mult)
                    conv = sm.tile([P, K], F32, tag="conv")
                    nc.vector.reduce_sum(conv[:], prod[:], axis=mybir.AxisListType.X)
                    nc.vector.tensor_add(conv[:], conv[:], wb_t[:, g, K * 9 :])
                    lr = sm.tile([P, K], F32, tag="lr")
                    nc.scalar.activation(lr[:], conv[:], AFT.Lrelu, alpha=NEG)
                    sig = sm.tile([P, K], F32, tag="sig")
                    gate = sm.tile([P, 1], F32, tag="gate")
                    nc.scalar.activation(sig[:], lr[:], AFT.Sigmoid, accum_out=gate[:])
                    s = sp8.tile([P, 1], F32, tag="s")
                    nc.scalar.activation(s[:], gate[:], AFT.Lrelu, alpha=NEG)
                    return s

                for r in range(repeat):
                    arena_b = {}
                    s_tl = {}
                    for i in range(B_SH * G):
                        at = ab.tile([P, HW], BF16, tag="at")
                        arena_b[i] = at
                        nc.gpsimd.dma_start(at[:], x2[i])
                        xv = at[:].rearrange(
                            "p (hb h wb w) -> p hb wb h w", hb=3, h=32, wb=3, w=32
                        )
                        pt = sm.tile([P, 9], F32, tag="pooled")
                        nc.vector.reduce_max(
                            pt[:].rearrange("p (hb wb) -> p hb wb", hb=3),
                            xv,
                            axis=mybir.AxisListType.XY,
                        )
                        s_tl[i] = gate_chain_b(i % G, pt)
                    for i in range(B_SH * G):
                        at = arena_b.pop(i)
                        nc.scalar.activation(
                            at[:], at[:], AFT.Lrelu, scale=s_tl[i][:], alpha=NEG
                        )
                        nc.gpsimd.dma_start(o2[i], at[:])
                seq = []

            if mode == "bf16store":
                # SWDGE cast-store probe: SBUF bf16 -> HBM f32, 24 units/pass
                o3 = out.rearrange(
                    "b (g p) (hb h) w -> b g hb p (h w)", g=G, p=P, hb=3, h=32
                )
                stb = cst.tile([P, 3072], BF16)
                nc.vector.memset(stb[:], 0.25)
                units = [
                    (b, g, hb) for b in range(B_SH) for g in range(G) for hb in range(3)
                ]
                for r in range(repeat):
                    for (b, g, hb) in units:
                        nc.gpsimd.dma_start(o3[b, g, hb], stb[:])
                seq = []

            if mode == "lsconst":
                # phased-bus probe: [4 loads][4 const stores], no cross-deps
                st = cst.tile([P, HW], F32)
                nc.scalar.dma_start(st[:], x2[0])
                slots = [cst.tile([P, HW], F32, name=f"slot{k}") for k in range(4)]
                for base in range(0, len(seq), 4):
                    for k in range(4):
                        nc.sync.dma_start(slots[k][:], x2[seq[base + k]])
                    for k in range(4):
                        nc.sync.dma_start(o2[seq[base + k]], st[:])
                seq = []

            def load(j):
                xt = xp.tile([P, HW], F32, tag="xt")
                if mode == "old":
                    # legacy schedule: loads on the ACT HWDGE queue
                    nc.scalar.dma_start(xt[:], x2[seq[j]])
                else:
                    nc.sync.dma_start(xt[:], x2[seq[j]])
                xts[j] = xt

            def compute_store(j):
                i = seq[j]
                g = i % G
                xt = xts.pop(j)
                if mode == "load":
                    # load-only probe: consume the tile with a tiny store
                    nc.sync.dma_start(o2[i, :, 0:16], xt[:, 0:16])
                    return
                if mode == "copy":
                    nc.sync.dma_start(o2[i], xt[:])
                    return
                # 32x32 block max: view [p, hb, wb, h, w], reduce (h, w)
                xv = xt[:].rearrange(
                    "p (hb h wb w) -> p hb wb h w", hb=3, h=32, wb=3, w=32
                )
                pooled = sm.tile([P, 9], F32, tag="pooled")
                nc.vector.reduce_max(
                    pooled[:].rearrange("p (hb wb) -> p hb wb", hb=3),
                    xv,
                    axis=mybir.AxisListType.XY,
                )

                # conv[p,k] = sum_j pooled[p,j] * wt[p,k,j]  (+ bias)
                prod = sm.tile([P, K, 9], F32, tag="prod")
                pooled_b = pooled[:].unsqueeze(1).broadcast_to([P, K, 9])
                wt_v = wb_t[:, g, 0 : K * 9].rearrange("p (k n) -> p k n", k=K)
                nc.vector.tensor_tensor(prod[:], wt_v, pooled_b, ALU.mult)
                conv = sm.tile([P, K], F32, tag="conv")
                nc.vector.reduce_sum(conv[:], prod[:], axis=mybir.AxisListType.X)
                nc.vector.tensor_add(conv[:], conv[:], wb_t[:, g, K * 9 :])

                # gate = sum_k sigmoid(leaky(conv)); scale = leaky(gate)
                lr = sm.tile([P, K], F32, tag="lr")
                nc.scalar.activation(lr[:], conv[:], AFT.Lrelu, alpha=NEG)
                sig = sm.tile([P, K], F32, tag="sig")
                gate = sm.tile([P, 1], F32, tag="gate")
                nc.scalar.activation(sig[:], lr[:], AFT.Sigmoid, accum_out=gate[:])
                s = sm.tile([P, 1], F32, tag="s")
                nc.scalar.activation(s[:], gate[:], AFT.Lrelu, alpha=NEG)

                # out = leaky(s * x), in place on the big tile
                nc.scalar.activation(xt[:], xt[:], AFT.Lrelu, scale=s[:], alpha=NEG)
                nc.sync.dma_start(o2[i], xt[:])

            # direction-phased schedule: emit PH loads, then their PH
            # compute+stores; SP-queue FIFO keeps the HBM bus uni-directional
            # within each phase. The fence (a tiny DMA that reads the LAST
            # load of the phase) blocks the store phase until every SDMA
            # engine has drained the whole load phase, re-aligning the 16
            # independent engine rings at each direction switch.
            PASS = B_SH * G

            def cross_core_sync(dep_i, dge):
                # stores(dep_i) -> bar_in copy -> AllReduce -> bar_out read
                # on the load queue: FIFO order gates the next pass's loads
                # until every core has finished the current pass.
                nc.sync.dma_start(bar_in[:], o2[dep_i][0:1, 0:2])
                nc.gpsimd.collective_compute(
                    "AllReduce",
                    ALU.add,
                    replica_groups=[list(range(N_CORES))],
                    ins=[bar_in[:].opt()],
                    outs=[bar_out[:].opt()],
                )
                f3 = sm.tile([1, 2], F32, tag="f3")
                dge.dma_start(f3[:], bar_out[:])

            if mode in ("old", "barrierold"):
                # legacy deep-pipeline schedule (loads 5 ahead of stores)
                OLD_DEPTH = 4  # pool has 4 bufs in the phased layout
                for j in range(min(OLD_DEPTH, len(seq))):
                    load(j)
                for j in range(len(seq)):
                    if mode == "barrierold" and j % PASS == 0 and j > 0:
                        cross_core_sync(seq[j - 1], nc.scalar)
                    compute_store(j)
                    if j + OLD_DEPTH < len(seq):
                        load(j + OLD_DEPTH)
                seq = []

            fence_on = mode in ("fence", "barrier")
            prev_last = None
            for base in range(0, len(seq), PH):
                hi = min(base + PH, len(seq))
                if mode == "barrier" and base % PASS == 0 and base > 0:
                    cross_core_sync(seq[base - 1], nc.sync)
                if fence_on and prev_last is not None:
                    # S->L fence: depend on the previous phase's last store
                    # (8B DRAM read of its output region)
                    f2 = sm.tile([1, 2], F32, tag="fence2")
                    nc.sync.dma_start(f2[:], o2[prev_last][0:1, 0:2])
                for j in range(base, hi):
                    load(j)
                if fence_on:
                    # L->S fence: depend on this phase's last load
                    f1 = sm.tile([1, 1], F32, tag="fence1")
                    nc.sync.dma_start(f1[:], xts[hi - 1][0:1, 0:1])
                for j in range(base, hi):
                    compute_store(j)
                prev_last = seq[hi - 1]
    nc.finalize()
    return nc


def _prep_small(w: np.ndarray, b: np.ndarray):
    # wb[p, g, k*9 + i*3 + j] = w[k, g*128+p, i, j]; wb[p, g, 144+k] = b[k, g*128+p]
    wt = w.transpose(1, 0, 2, 3).reshape(G, P, K * 9).transpose(1, 0, 2)
    bt = b.T.reshape(G, P, K).transpose(1, 0, 2)
    return np.ascontiguousarray(np.concatenate([wt, bt], axis=2))


def run(inputs: dict, trace: bool = False, mode: str = "fence"):
    x = np.ascontiguousarray(np.asarray(inputs["x"], dtype=np.float32))
    w = np.asarray(inputs["w"], dtype=np.float32)
    b = np.asarray(inputs["b"], dtype=np.float32)
    wb = _prep_small(w, b)

    nc = build(mode=mode)
    in_maps = [
        {"x": np.ascontiguousarray(x[i * B_SH : (i + 1) * B_SH]), "wb": wb}
        for i in range(N_CORES)
    ]
    res = run_bass_kernel_spmd(nc, in_maps, core_ids=list(range(N_CORES)), trace=trace)
    out = np.concatenate([r["out"] for r in res.results], axis=0)
    return out, res


def kernel(**inputs) -> np.ndarray:
    out, _ = run(inputs, trace=False)
    return out



# revision 33
# speedup vs baseline: 1.0125x; 1.0125x over previous
"""Trainium2 Bass kernel for nn_Channel: adaptive max-pool(3) -> 16 depthwise
3x3 convs -> sigmoid-sum channel gate -> leaky(gate*x).

Data-parallel over batch: 32 batches -> 4 per core x 8 cores. Weights/biases
replicated. Self-contained: hardcodes shapes from the problem spec.

The kernel is HBM-bandwidth bound (read x once + write out once = 75.5 MB
per core). Default schedule ("fence"): every DMA is issued on the single SP
HWDGE queue (qSPDynamicHW) in [4 loads][4 stores] phase order — FIFO on one
queue keeps the HBM bus direction-pure within each phase (measured: mixed
read/write traffic runs 10-20% slower than phase-separated). Tiny fence DMAs
at each phase boundary re-align the 16 independent SDMA engine rings so fast
engines cannot run ahead into the opposite-direction phase.
"""

import numpy as np

import concourse.bacc as bacc
import concourse.tile as tile
from concourse import mybir
from concourse.bass_utils import run_bass_kernel_spmd

AFT = mybir.ActivationFunctionType
ALU = mybir.AluOpType
F32 = mybir.dt.float32
BF16 = mybir.dt.bfloat16

B, C, H, W = 32, 256, 96, 96
N_CORES = 8
B_SH = B // N_CORES          # 4 batches per core
P = 128                      # SBUF partitions
G = C // P                   # 2 channel groups
HW = H * W                   # 9216
K = 16                       # number of depthwise convs
NEG = 0.01                   # leaky relu slope (torch default)


def build(repeat: int = 1, copy_only: bool = False, mode: str = "full"):
    if copy_only:
        mode = "copy"
    nc = bacc.Bacc(None, num_devices=N_CORES if "barrier" in mode else None)
    x = nc.dram_tensor("x", [B_SH, C, H, W], F32, kind="ExternalInput")
    if "barrier" in mode:
        bar_in = nc.dram_tensor("bar_in", [1, 2], F32)
        bar_out = nc.dram_tensor("bar_out", [1, 2], F32)
    # packed per-channel weights+biases: [p, g, k*9 weights .. k biases]
    wb = nc.dram_tensor("wb", [P, G, K * 9 + K], F32, kind="ExternalInput")
    out = nc.dram_tensor("out", [B_SH, C, H, W], F32, kind="ExternalOutput")

    # channel c = g*128 + p -> partition p of group g
    x2 = x.rearrange("b (g p) h w -> (b g) p (h w)", g=G, p=P)
    o2 = out.rearrange("b (g p) h w -> (b g) p (h w)", g=G, p=P)

    # All DMAs go through the single SP HWDGE queue (qSPDynamicHW) in
    # program order: 4 loads, then 4 stores, alternating. One FIFO queue
    # means the SDMA engines never interleave read and write packets on
    # the HBM bus (measured: mixed-direction traffic runs ~13% slower
    # than phase-separated; load-only 371 GB/s, store-only 379 GB/s).
    PH = 4  # tiles per direction phase; also the xp pool depth (4x36KB/part)
    DEPTH = PH

    with tile.TileContext(nc) as tc:
        with (
            tc.tile_pool(name="xp", bufs=DEPTH) as xp,
            tc.tile_pool(name="cst", bufs=1) as cst,
            tc.tile_pool(name="sm", bufs=4) as sm,
            tc.tile_pool(name="sp8", bufs=8) as sp8,
            tc.tile_pool(name="ab", bufs=8 if mode == "bf16b" else 24) as ab,
        ):
            wb_t = cst.tile([P, G, K * 9 + K], F32)
            # wb rides the otherwise-idle ACT HWDGE queue so it does not
            # delay the first x load at the head of the SP FIFO
            (nc.scalar if mode in ("fence", "fence1") else nc.sync).dma_start(
                wb_t[:], wb[:]
            )
            # warmup read so the wb DMA wait lands here, not on the first
            # TensorTensor (whose ISA format has too few sync-wait slots)
            warm = cst.tile([P, 1], F32)
            nc.vector.tensor_copy(warm[:], wb_t[:, 0, 0:1])

            # seq[j] = tile index of the j-th unit of work; repeat>1 re-runs
            # the whole pass (for differential HW timing) writing identical
            # bytes to out each pass.
            seq = [i % (B_SH * G) for i in range(B_SH * G * repeat)]
            xts = {}

            if mode == "store":
                # store-only probe: one tile loaded once, stored everywhere
                st = cst.tile([P, HW], F32)
                nc.scalar.dma_start(st[:], x2[0])
                for i in seq:
                    nc.sync.dma_start(o2[i], st[:])
                seq = []

            if mode == "bf16load":
                # SWDGE cast-load probe: HBM f32 -> SBUF bf16, 24 units/pass
                x3 = x.rearrange(
                    "b (g p) (hb h) w -> b g hb p (h w)", g=G, p=P, hb=3, h=32
                )
                aslots = [cst.tile([P, 3072], BF16, name=f"a{k}") for k in range(6)]
                units = [
                    (b, g, hb) for b in range(B_SH) for g in range(G) for hb in range(3)
                ]
                for r in range(repeat):
                    for u, (b, g, hb) in enumerate(units):
                        nc.gpsimd.dma_start(aslots[u % 6][:], x3[b, g, hb])
                seq = []

            if mode == "bf16":
                # bf16-arena schedule: the whole pass (24 units of
                # [128, 3072]) lives in SBUF as bf16, so each pass is ONE
                # load phase + ONE store phase. Every big transfer is a
                # SWDGE cast DMA (f32 HBM <-> bf16 SBUF) on the single
                # qPoolDynamic queue, whose FIFO order alone enforces
                # direction-pure phases. bf16 rounding of x and of the
                # scaled output adds ~1e-3 rel error (gate <2e-2 tol).
                x3 = x.rearrange(
                    "b (g p) (hb h) w -> b g hb p (h w)", g=G, p=P, hb=3, h=32
                )
                o3 = out.rearrange(
                    "b (g p) (hb h) w -> b g hb p (h w)", g=G, p=P, hb=3, h=32
                )
                units = [
                    (b, g, hb) for b in range(B_SH) for g in range(G) for hb in range(3)
                ]

                def gate_chain(g, pt):
                    # conv[p,k] = sum_j pooled[p,j] * wt[p,k,j] + bias
                    prod = sm.tile([P, K, 9], F32, tag="prod")
                    pooled_b = pt[:].unsqueeze(1).broadcast_to([P, K, 9])
                    wt_v = wb_t[:, g, 0 : K * 9].rearrange("p (k n) -> p k n", k=K)
                    nc.vector.tensor_tensor(prod[:], wt_v, pooled_b, ALU.mult)
                    conv = sm.tile([P, K], F32, tag="conv")
                    nc.vector.reduce_sum(conv[:], prod[:], axis=mybir.AxisListType.X)
                    nc.vector.tensor_add(conv[:], conv[:], wb_t[:, g, K * 9 :])
                    lr = sm.tile([P, K], F32, tag="lr")
                    nc.scalar.activation(lr[:], conv[:], AFT.Lrelu, alpha=NEG)
                    sig = sm.tile([P, K], F32, tag="sig")
                    gate = sm.tile([P, 1], F32, tag="gate")
                    nc.scalar.activation(sig[:], lr[:], AFT.Sigmoid, accum_out=gate[:])
                    s = sp8.tile([P, 1], F32, tag="s")
                    nc.scalar.activation(s[:], gate[:], AFT.Lrelu, alpha=NEG)
                    return s

                for r in range(repeat):
                    arena = {}
                    pooled = {}
                    s_tiles = {}
                    for (b, g, hb) in units:
                        at = ab.tile([P, 3072], BF16, tag="at")
                        arena[(b, g, hb)] = at
                        nc.gpsimd.dma_start(at[:], x3[b, g, hb])
                        if hb == 0:
                            pt = sm.tile([P, 9], F32, tag="pooled")
                            pooled[(b, g)] = pt
                        xv = at[:].rearrange("p (h wb w) -> p wb h w", h=32, wb=3, w=32)
                        nc.vector.reduce_max(
                            pooled[(b, g)][:, hb * 3 : (hb + 1) * 3],
                            xv,
                            axis=mybir.AxisListType.XY,
                        )
                        if hb == 2:
                            s_tiles[(b, g)] = gate_chain(g, pooled[(b, g)])
                    for (b, g, hb) in units:
                        at = arena.pop((b, g, hb))
                        nc.scalar.activation(
                            at[:], at[:], AFT.Lrelu, scale=s_tiles[(b, g)][:], alpha=NEG
                        )
                        nc.gpsimd.dma_start(o3[b, g, hb], at[:])
                seq = []

            if mode == "bf16b":
                # bf16-arena with full-size units: 8 tiles of [128, 9216]
                # bf16 per pass (one SWDGE cast DMA each way per tile, one
                # descriptor per partition row). Same phase structure as
                # "bf16" but a third of the DMA/instruction count.
                def gate_chain_b(g, pt):
                    prod = sm.tile([P, K, 9], F32, tag="prod")
                    pooled_b = pt[:].unsqueeze(1).broadcast_to([P, K, 9])
                    wt_v = wb_t[:, g, 0 : K * 9].rearrange("p (k n) -> p k n", k=K)
                    nc.vector.tensor_tensor(prod[:], wt_v, pooled_b, ALU.mult)
                    conv = sm.tile([P, K], F32, tag="conv")
                    nc.vector.reduce_sum(conv[:], prod[:], axis=mybir.AxisListType.X)
                    nc.vector.tensor_add(conv[:], conv[:], wb_t[:, g, K * 9 :])
                    lr = sm.tile([P, K], F32, tag="lr")
                    nc.scalar.activation(lr[:], conv[:], AFT.Lrelu, alpha=NEG)
                    sig = sm.tile([P, K], F32, tag="sig")
                    gate = sm.tile([P, 1], F32, tag="gate")
                    nc.scalar.activation(sig[:], lr[:], AFT.Sigmoid, accum_out=gate[:])
                    s = sp8.tile([P, 1], F32, tag="s")
                    nc.scalar.activation(s[:], gate[:], AFT.Lrelu, alpha=NEG)
                    return s

                for r in range(repeat):
                    arena_b = {}
                    s_tl = {}
                    for i in range(B_SH * G):
                        at = ab.tile([P, HW], BF16, tag="at")
                        arena_b[i] = at
                        nc.gpsimd.dma_start(at[:], x2[i])
                        xv = at[:].rearrange(
                            "p (hb h wb w) -> p hb wb h w", hb=3, h=32, wb=3, w=32
                        )
                        pt = sm.tile([P, 9], F32, tag="pooled")
                        nc.vector.reduce_max(
                            pt[:].rearrange("p (hb wb) -> p hb wb", hb=3),
                            xv,
                            axis=mybir.AxisListType.XY,
                        )
                        s_tl[i] = gate_chain_b(i % G, pt)
                    for i in range(B_SH * G):
                        at = arena_b.pop(i)
                        nc.scalar.activation(
                            at[:], at[:], AFT.Lrelu, scale=s_tl[i][:], alpha=NEG
                        )
                        nc.gpsimd.dma_start(o2[i], at[:])
                seq = []

            if mode == "bf16store":
                # SWDGE cast-store probe: SBUF bf16 -> HBM f32, 24 units/pass
                o3 = out.rearrange(
                    "b (g p) (hb h) w -> b g hb p (h w)", g=G, p=P, hb=3, h=32
                )
                stb = cst.tile([P, 3072], BF16)
                nc.vector.memset(stb[:], 0.25)
                units = [
                    (b, g, hb) for b in range(B_SH) for g in range(G) for hb in range(3)
                ]
                for r in range(repeat):
                    for (b, g, hb) in units:
                        nc.gpsimd.dma_start(o3[b, g, hb], stb[:])
                seq = []

            if mode == "lsconst":
                # phased-bus probe: [4 loads][4 const stores], no cross-deps
                st = cst.tile([P, HW], F32)
                nc.scalar.dma_start(st[:], x2[0])
                slots = [cst.tile([P, HW], F32, name=f"slot{k}") for k in range(4)]
                for base in range(0, len(seq), 4):
                    for k in range(4):
                        nc.sync.dma_start(slots[k][:], x2[seq[base + k]])
                    for k in range(4):
                        nc.sync.dma_start(o2[seq[base + k]], st[:])
                seq = []

            def load(j):
                xt = xp.tile([P, HW], F32, tag="xt")
                if mode == "old":
                    # legacy schedule: loads on the ACT HWDGE queue
                    nc.scalar.dma_start(xt[:], x2[seq[j]])
                else:
                    nc.sync.dma_start(xt[:], x2[seq[j]])
                xts[j] = xt

            def compute_store(j):
                i = seq[j]
                g = i % G
                xt = xts.pop(j)
                if mode == "load":
                    # load-only probe: consume the tile with a tiny store
                    nc.sync.dma_start(o2[i, :, 0:16], xt[:, 0:16])
                    return
                if mode == "copy":
                    nc.sync.dma_start(o2[i], xt[:])
                    return
                # 32x32 block max: view [p, hb, wb, h, w], reduce (h, w)
                xv = xt[:].rearrange(
                    "p (hb h wb w) -> p hb wb h w", hb=3, h=32, wb=3, w=32
                )
                pooled = sm.tile([P, 9], F32, tag="pooled")
                nc.vector.reduce_max(
                    pooled[:].rearrange("p (hb wb) -> p hb wb", hb=3),
                    xv,
                    axis=mybir.AxisListType.XY,
                )

                # conv[p,k] = sum_j pooled[p,j] * wt[p,k,j]  (+ bias)
                prod = sm.tile([P, K, 9], F32, tag="prod")
                pooled_b = pooled[:].unsqueeze(1).broadcast_to([P, K, 9])
                wt_v = wb_t[:, g, 0 : K * 9].rearrange("p (k n) -> p k n", k=K)
                nc.vector.tensor_tensor(prod[:], wt_v, pooled_b, ALU.mult)
                conv = sm.tile([P, K], F32, tag="conv")
                nc.vector.reduce_sum(conv[:], prod[:], axis=mybir.AxisListType.X)
                nc.vector.tensor_add(conv[:], conv[:], wb_t[:, g, K * 9 :])

                # gate = sum_k sigmoid(leaky(conv)); scale = leaky(gate)
                lr = sm.tile([P, K], F32, tag="lr")
                nc.scalar.activation(lr[:], conv[:], AFT.Lrelu, alpha=NEG)
                sig = sm.tile([P, K], F32, tag="sig")
                gate = sm.tile([P, 1], F32, tag="gate")
                nc.scalar.activation(sig[:], lr[:], AFT.Sigmoid, accum_out=gate[:])
                s = sm.tile([P, 1], F32, tag="s")
                nc.scalar.activation(s[:], gate[:], AFT.Lrelu, alpha=NEG)

                # out = leaky(s * x), in place on the big tile
                nc.scalar.activation(xt[:], xt[:], AFT.Lrelu, scale=s[:], alpha=NEG)
                nc.sync.dma_start(o2[i], xt[:])

            # direction-phased schedule: emit PH loads, then their PH
            # compute+stores; SP-queue FIFO keeps the HBM bus uni-directional
            # within each phase. The fence (a tiny DMA that reads the LAST
            # load of the phase) blocks the store phase until every SDMA
            # engine has drained the whole load phase, re-aligning the 16
            # independent engine rings at each direction switch.
            PASS = B_SH * G

            def cross_core_sync(dep_i, dge):
                # stores(dep_i) -> bar_in copy -> AllReduce -> bar_out read
                # on the load queue: FIFO order gates the next pass's loads
                # until every core has finished the current pass.
                nc.sync.dma_start(bar_in[:], o2[dep_i][0:1, 0:2])
                nc.gpsimd.collective_compute(
                    "AllReduce",
                    ALU.add,
                    replica_groups=[list(range(N_CORES))],
                    ins=[bar_in[:].opt()],
                    outs=[bar_out[:].opt()],
                )
                f3 = sm.tile([1, 2], F32, tag="f3")
                dge.dma_start(f3[:], bar_out[:])

            if mode in ("old", "barrierold"):
                # legacy deep-pipeline schedule (loads 5 ahead of stores)
                OLD_DEPTH = 4  # pool has 4 bufs in the phased layout
                for j in range(min(OLD_DEPTH, len(seq))):
                    load(j)
                for j in range(len(seq)):
                    if mode == "barrierold" and j % PASS == 0 and j > 0:
                        cross_core_sync(seq[j - 1], nc.scalar)
                    compute_store(j)
                    if j + OLD_DEPTH < len(seq):
                        load(j + OLD_DEPTH)
                seq = []

            fence_on = mode in ("fence", "fence1", "barrier")
            prev_last = None
            for base in range(0, len(seq), PH):
                hi = min(base + PH, len(seq))
                if mode == "barrier" and base % PASS == 0 and base > 0:
                    cross_core_sync(seq[base - 1], nc.sync)
                if mode != "fence1" and fence_on and prev_last is not None:
                    # S->L fence: depend on the previous phase's last store
                    # (8B DRAM read of its output region)
                    f2 = sm.tile([1, 2], F32, tag="fence2")
                    nc.sync.dma_start(f2[:], o2[prev_last][0:1, 0:2])
                for j in range(base, hi):
                    load(j)
                if fence_on:
                    # L->S fence: depend on this phase's last load
                    f1 = sm.tile([1, 1], F32, tag="fence1")
                    nc.sync.dma_start(f1[:], xts[hi - 1][0:1, 0:1])
                for j in range(base, hi):
                    compute_store(j)
                prev_last = seq[hi - 1]
    nc.finalize()
    return nc


def _prep_small(w: np.ndarray, b: np.ndarray):
    # wb[p, g, k*9 + i*3 + j] = w[k, g*128+p, i, j]; wb[p, g, 144+k] = b[k, g*128+p]
    wt = w.transpose(1, 0, 2, 3).reshape(G, P, K * 9).transpose(1, 0, 2)
    bt = b.T.reshape(G, P, K).transpose(1, 0, 2)
    return np.ascontiguousarray(np.concatenate([wt, bt], axis=2))


def run(inputs: dict, trace: bool = False, mode: str = "fence"):
    x = np.ascontiguousarray(np.asarray(inputs["x"], dtype=np.float32))
    w = np.asarray(inputs["w"], dtype=np.float32)
    b = np.asarray(inputs["b"], dtype=np.float32)
    wb = _prep_small(w, b)

    nc = build(mode=mode)
    in_maps = [
        {"x": np.ascontiguousarray(x[i * B_SH : (i + 1) * B_SH]), "wb": wb}
        for i in range(N_CORES)
    ]
    res = run_bass_kernel_spmd(nc, in_maps, core_ids=list(range(N_CORES)), trace=trace)
    out = np.concatenate([r["out"] for r in res.results], axis=0)
    return out, res


def kernel(**inputs) -> np.ndarray:
    out, _ = run(inputs, trace=False)
    return out

